# revision 26
# baseline (speedup 1.0000x reference)
"""Trainium2 Bass kernel v6 for nn_FE_block_3d: four 3D-conv branches (32->8
ch, 3x3x3, SAME) over different triples of the (U,V,H,W) dims of
x [8, 32, 5, 5, 64, 64], each followed by bias + PReLU, concatenated on the
channel axis.

v6 changes vs v5 (attacking the DMA per-packet-overhead bound: each DMA
queue moves ~1 packet / 55ns, packet = per-partition contiguous bytes, so
the old 2.9KB xv packets capped each queue at ~52 GB/s and xv demand alone
saturated both HWDGEs):
- B/C staging drops the 5-row overlapping W-window (1.72x replication) for
  3 residue rows (w = 3*wbi + r - 1): K = (c8, s5, r3) = 120, exact c-chunks
  of 8, taps split over two wb-shifts (delta in {0,1}) -> 24 accum matmuls
  per PSUM group (+14% B/C columns) but xv DMA drops 23.2 -> 13.4 MB/rep.
- xb/xc DRAM staged s-minor per partition; one DMA loads an s-pair with
  6KB packets (~110 GB/s/queue).
- outB/outC written in one whole-phase DMA (14KB packets), wA in one DMA.
- DGE routing: gpsimd SWDGE carries xa bulk; SP/ACT HWDGEs split xv/outs.
Carried from v3-v5: bf16 everywhere, group-sequential PSUM accumulation,
persistent pools, h-blocked phase A groups (contiguous rhs runs).

v7 (attacking the single-shot / cold-start path; steady state was already
100% PE-busy at full clock, bound by matmul rows ~585K/rep = ~256us at
2.4GHz — density redesigns all dead-end, see session notes):
- In-rep phase order B, C, A: phase A's 12MB of xa/wA bulk now has the
  full ~146us of B+C compute to prefetch, instead of stalling the first
  ~32us of phase A matmuls (rep0 trace gaps).
- wA/xa DMAs issued after B's last s-pair loads are enqueued (pi==2) so
  the A bulk doesn't steal cold-start DMA bandwidth from B's own xv
  stream; still ~110us of lead before A consumes them.
- First s-pair is a single fold (SPAIRS (0,1),(1,2),(3,2)) and its four
  c-chunk tiles are split across both HWDGE families, halving the
  first-group data latency at NEFF start.
- outA written in h-halves at g==3/g==7; vc0 rides gpsimd SWDGE
  (mid-rep), vc1 the by-then-idle HWDGEs, so the end-of-NEFF tail is a
  single ~0.3MB DMA instead of 17us of drains.
"""

import sys

if "/opt/trn_rl_repo" not in sys.path:
    sys.path.insert(0, "/opt/trn_rl_repo")

import numpy as np
import ml_dtypes

BF16 = ml_dtypes.bfloat16

C_IN = 32
U = V = 5
H = W = 64
KS = 3
N_CORES = 8

_NC_CACHE = {}

COMPACT_BC = True


# ---------------------------------------------------------------------------
# Host-side data staging
# ---------------------------------------------------------------------------

def _tap_matrix(n):
    t = np.zeros((KS, n, n), np.float32)
    for i in range(n):
        for ip in range(n):
            k = i - ip + 1
            if 0 <= k < KS:
                t[k, i, ip] = 1.0
    return t


VC_A = (
    dict(vps=(0, 1, 2), vs=(0, 1, 2, 3), csz=6, ncc=6),
    dict(vps=(3, 4), vs=(2, 3, 4), csz=8, ncc=4),
)

HBLOCKS = ((0, 22), (22, 21), (43, 21))
SPAIRS = ((0, 1), (1, 2), (3, 2))
NCC_BC = 4


def _build_wA(w_uvx, w_uvy):
    T5 = _tap_matrix(5)
    out = np.zeros((60, 120, 120), np.float32)
    for br, w in enumerate((w_uvx, w_uvy)):
        for vc, cfg in enumerate(VC_A):
            vps, vs, csz, ncc = cfg["vps"], cfg["vs"], cfg["csz"], cfg["ncc"]
            nv, nvp = len(vs), len(vps)
            Tv = np.zeros((KS, nv, nvp), np.float32)
            for vi, v in enumerate(vs):
                for vpi, vp in enumerate(vps):
                    kv = v - vp + 1
                    if 0 <= kv < KS:
                        Tv[kv, vi, vpi] = 1.0
            for tap in range(KS):
                full = np.einsum("ocab,auy,bvz->cuvoyz",
                                 w[:, :, :, :, tap], T5, Tv)
                for cc in range(ncc):
                    c0 = csz * cc
                    cn = min(csz, 32 - c0)
                    n = br * 30 + (tap * 6 + cc if vc == 0
                                   else 18 + tap * 4 + cc)
                    blk = full[c0:c0 + cn]
                    out[n, :cn * 5 * nv, :8 * 5 * nvp] = blk.reshape(
                        cn * 5 * nv, 8 * 5 * nvp)
    return np.ascontiguousarray(
        out.transpose(1, 0, 2).reshape(120, 7200)).astype(BF16)


def _build_wBC(w):
    """[120, 24*120] stationary mats for phases B/C; n = kh*8 + delta*4 + cc.
    k = c_l*15 + u*3 + r (c_l < 8, r = w-residue row), m = o*15 + s'*3 + jw.
    Taps: kw = 3*delta + r - jw; rhs applies wb-offset delta in {0,1}."""
    T5 = _tap_matrix(5)
    Rd = np.zeros((2, KS, 3, 3), np.float32)  # [delta, kw, r, jw]
    for d in range(2):
        for r in range(3):
            for jw in range(3):
                kw = 3 * d + r - jw
                if 0 <= kw < KS:
                    Rd[d, kw, r, jw] = 1.0
    out = np.zeros((24, 120, 120), np.float32)
    for kh in range(KS):
        for d in range(2):
            for cc in range(NCC_BC):
                wsl = w[:, 8 * cc:8 * cc + 8, :, kh, :]  # [o, c8, ks, kw]
                sub = np.einsum("ocab,auy,brj->curoyj", wsl, T5, Rd[d])
                out[kh * 8 + d * 4 + cc] = sub.reshape(120, 120)
    return np.ascontiguousarray(
        out.transpose(1, 0, 2).reshape(120, 24 * 120)).astype(BF16)


def _build_bias(biases, alphas):
    out = np.zeros((128, 18), np.float32)
    bA = [np.asarray(b, np.float32) for b in biases]
    units = [
        (np.repeat(bA[0], 15), alphas[0]),   # uvx vc0: m=(o,u',v'3)
        (np.repeat(bA[0], 10), alphas[0]),   # uvx vc1: m=(o,u',v'2)
        (np.repeat(bA[1], 15), alphas[1]),   # uvy vc0
        (np.repeat(bA[1], 10), alphas[1]),   # uvy vc1
        (np.repeat(bA[2], 15), alphas[2]),   # uxy: m=(o,u',jw)
        (np.repeat(bA[3], 15), alphas[3]),   # vxy
    ]
    for i, (col, a) in enumerate(units):
        out[: col.size, 3 * i] = col
        out[: col.size, 3 * i + 1] = float(a) * col
        out[: col.size, 3 * i + 2] = float(a)
    return out


def _prep_xa(x):
    xa1 = np.zeros((6, 120, 66, 66), BF16)
    xa2 = np.zeros((4, 120, 66, 66), BF16)
    xb = x.astype(BF16)
    for cc in range(6):
        c0 = 6 * cc
        cn = min(6, 32 - c0)
        blk = xb[c0:c0 + cn][:, :, 0:4]
        xa1[cc, :cn * 20, 1:65, 1:65] = blk.reshape(cn * 20, 64, 64)
    for cc in range(4):
        blk = xb[8 * cc:8 * cc + 8][:, :, 2:5]
        xa2[cc, :, 1:65, 1:65] = blk.reshape(120, 64, 64)
    return xa1, xa2


def _prep_xbc(x, phase):
    """x [32,5,5,64,64] -> [4 (cc), 120, 5 (fold, s-minor), 66, 23] bf16.
    Partition k = c_l*15 + s*3 + r with s = u (phase 0) or v (phase 1) and r
    the W-residue row (w = 3*wbi + r - 1, zero-padded); free dims per
    partition are s-minor so one DMA spans several folds contiguously."""
    xs = x if phase == 0 else np.ascontiguousarray(x.transpose(0, 2, 1, 3, 4))
    xpad = np.zeros((32, 5, 5, 64, 70), np.float32)
    xpad[..., 1:65] = xs
    E = np.stack([xpad[..., r::3][..., :23] for r in range(3)], axis=-1)
    # E: (c, d1, d2, h, wbi, r); fold = d2, banded s-axis = d1
    E2 = E.transpose(0, 1, 5, 2, 3, 4)  # (c, d1, r, fold, h, wbi)
    out = np.zeros((4, 120, 5, 66, 23), BF16)
    for cc in range(NCC_BC):
        blk = E2[8 * cc:8 * cc + 8]  # (c8, s5, r3, fold5, h64, wbi23)
        out[cc, :, :, 1:65, :] = blk.reshape(120, 5, 64, 23)
    return out


# ---------------------------------------------------------------------------
# Bass kernel construction
# ---------------------------------------------------------------------------

def _build_nc(repeat=1):
    import concourse.bass as bass
    import concourse.mybir as mybir
    from concourse import bacc
    from concourse.tile import TileContext

    BF = mybir.dt.bfloat16
    F32 = mybir.dt.float32
    ALU = mybir.AluOpType
    AF = mybir.ActivationFunctionType

    nc = bacc.Bacc("TRN2", target_bir_lowering=False)
    xa1_d = nc.dram_tensor("xa1", [6, 120, 66 * 66], BF, kind="ExternalInput")
    xa2_d = nc.dram_tensor("xa2", [4, 120, 66 * 66], BF, kind="ExternalInput")
    xb_d = nc.dram_tensor("xb", [4, 120, 5 * 66 * 23], BF, kind="ExternalInput")
    xc_d = nc.dram_tensor("xc", [4, 120, 5 * 66 * 23], BF, kind="ExternalInput")
    wA_d = nc.dram_tensor("wA", [120, 7200], BF, kind="ExternalInput")
    wB_d = nc.dram_tensor("wB", [120, 2880], BF, kind="ExternalInput")
    wC_d = nc.dram_tensor("wC", [120, 2880], BF, kind="ExternalInput")
    b_d = nc.dram_tensor("bias", [128, 18], F32, kind="ExternalInput")
    out_d = nc.dram_tensor("out", [16, U, V, H, W], BF, kind="ExternalOutput")
    outB_d = nc.dram_tensor("outB", [120, 5 * 64 * 22], BF, kind="ExternalOutput")
    outC_d = nc.dram_tensor("outC", [120, 5 * 64 * 22], BF, kind="ExternalOutput")
    dbg_d = nc.dram_tensor("dbg", [128, 4], F32, kind="ExternalOutput")

    PSUM = bass.MemorySpace.PSUM

    with TileContext(nc) as tc:
        with (
            tc.tile_pool(name="bias", bufs=1) as bias_pool,
            tc.tile_pool(name="warm", bufs=1, space=PSUM) as warm_pool,
            tc.tile_pool(name="wA", bufs=1) as wA_pool,
            tc.tile_pool(name="xa1", bufs=1) as xa1_pool,
            tc.tile_pool(name="xa2", bufs=1) as xa2_pool,
            tc.tile_pool(name="wbc", bufs=2) as w_pool,
            tc.tile_pool(name="xv", bufs=2) as xv_pool,
            tc.tile_pool(name="ps", bufs=6, space=PSUM) as ps_pool,
            tc.tile_pool(name="stgA", bufs=2) as stgA_pool,
            tc.tile_pool(name="stgB", bufs=2) as stgB_pool,
            tc.tile_pool(name="tq", bufs=3) as tq_pool,
        ):
            bias_t = bias_pool.tile([128, 18], F32)
            nc.gpsimd.dma_start(bias_t[:], b_d[:])
            # Persistent PSUM bank written only by PE "touch" matmuls. A PE
            # Matmult can carry at most ONE sync wait in walrus codegen, so
            # each freshly-DMA'd tile gets one touch matmul (1 wait each)
            # before the real accumulation groups consume it wait-free.
            warm_t = warm_pool.tile([128, 512], F32)
            for rep in range(repeat):
                # Phase A tiles are allocated up-front; their DMAs are
                # issued early in phase B so the 12MB of xa/wA bulk
                # streams on gpsimd/sync during the ~146us of B+C
                # compute (phase order in-rep is B, C, A).
                wA_t = wA_pool.tile([120, 60, 120], BF, tag="wA",
                                    name=f"wA{rep}")
                xa_tiles = {}
                for vc in range(2):
                    ncc = VC_A[vc]["ncc"]
                    xpool = xa1_pool if vc == 0 else xa2_pool
                    kps = [120] * 5 + [40] if vc == 0 else [120] * 4
                    for cc in range(ncc):
                        kp = kps[cc]
                        t = xpool.tile([kp, 66, 66], BF,
                                       tag=f"xa{vc}_{cc}",
                                       name=f"xa{rep}_{vc}{cc}")
                        xa_tiles[(vc, cc)] = (t, kp)

                def issue_a_loads():
                    nc.sync.dma_start(
                        wA_t[:], wA_d[:].rearrange("k (n m) -> k n m", m=120))
                    for vc in range(2):
                        xd = xa1_d if vc == 0 else xa2_d
                        for cc in range(VC_A[vc]["ncc"]):
                            t, kp = xa_tiles[(vc, cc)]
                            nc.gpsimd.dma_start(
                                t[:],
                                xd[cc, 0:kp].rearrange("p (h w) -> p h w",
                                                       w=66),
                            )

                # ---------------- Phases B (uxy) and C (vxy) ----------------
                # K = (c_l8, u-or-v, r3) = 120, M = (o, s', jw3) = 120,
                # 24 PSUM accums (kh x delta x 4 c-chunks) per h-block group.
                for phase in range(2):
                    xd = xb_d if phase == 0 else xc_d
                    wd = wB_d if phase == 0 else wC_d
                    od = outB_d if phase == 0 else outC_d
                    unit = 4 + phase
                    w_t = w_pool.tile([120, 24, 120], BF, tag="wbc",
                                      name=f"wmat{rep}_{phase}")
                    nc.scalar.dma_start(
                        w_t[:], wd[:].rearrange("k (n m) -> k n m", m=120)
                    )
                    nc.tensor.matmul(warm_t[0:120, 0:120], w_t[:, 0, :],
                                     w_t[:, 0, 0:120], start=True, stop=True)
                    ba = bias_t[0:120, 3 * unit + 2:3 * unit + 3]
                    bq = bias_t[0:120, 3 * unit + 1:3 * unit + 2]
                    bb = bias_t[0:120, 3 * unit:3 * unit + 1]
                    stg = stgB_pool.tile([120, 5, 64, 22], BF, tag="stgB",
                                         name=f"stgB{rep}_{phase}")
                    for pi, (s0, ns) in enumerate(SPAIRS):
                        xv = []
                        for cc in range(NCC_BC):
                            t = xv_pool.tile([120, 2, 66, 23], BF,
                                             tag=f"xv{cc}",
                                             name=f"xv{rep}_{phase}{pi}{cc}")
                            src = xd[cc, :, 1518 * s0:1518 * (s0 + ns)] \
                                .rearrange("p (s h w) -> p s h w",
                                           s=ns, w=23)
                            if rep == 0 and phase == 0 and pi == 0:
                                # cold start: halve each tile across both
                                # HWDGE families so the first group's four
                                # c-chunks land in ~half the single-queue
                                # drain time; touch each half (1 wait each).
                                nc.sync.dma_start(t[0:64, 0:ns], src[0:64])
                                nc.scalar.dma_start(t[64:120, 0:ns],
                                                    src[64:120])
                                nc.tensor.matmul(warm_t[0:120, 0:46],
                                                 w_t[0:64, 0, :],
                                                 t[0:64, 0, 0:2, :],
                                                 start=True, stop=True)
                                nc.tensor.matmul(warm_t[0:120, 0:46],
                                                 w_t[64:120, 0, :],
                                                 t[64:120, 0, 0:2, :],
                                                 start=True, stop=True)
                            else:
                                dge = nc.sync if cc % 2 == 0 else nc.scalar
                                dge.dma_start(t[:, 0:ns, :, :], src)
                                if cc == 0:
                                    nc.tensor.matmul(warm_t[0:120, 0:46],
                                                     w_t[:, 0, :],
                                                     t[:, 0, 0:2, :],
                                                     start=True, stop=True)
                            xv.append(t)
                        if phase == 0 and pi == 2:
                            # issue after B's own loads are enqueued so the
                            # 12MB of A bulk doesn't compete with the cold
                            # xv stream; still ~110us of lead before A.
                            issue_a_loads()
                        for sin in range(ns):
                            s = s0 + sin
                            for tg, (h0, hb) in enumerate(HBLOCKS):
                                ps = ps_pool.tile(
                                    [120, hb, 22], F32, tag="ps",
                                    name=f"psB{rep}_{phase}{s}{tg}")
                                for cc in range(NCC_BC):
                                    t = xv[cc]
                                    for kh in range(KS):
                                        for d in range(2):
                                            n = kh * 8 + d * 4 + cc
                                            rhs = t[0:120, sin,
                                                    h0 + kh:h0 + kh + hb,
                                                    d:d + 22]
                                            nc.tensor.matmul(
                                                ps[:], w_t[:, n, :], rhs,
                                                start=(cc == 0 and kh == 0
                                                       and d == 0),
                                                stop=(cc == NCC_BC - 1
                                                      and kh == 2 and d == 1),
                                            )
                                tq = tq_pool.tile(
                                    [120, hb, 22], F32, tag="tq",
                                    name=f"tqB{rep}_{phase}{s}{tg}")
                                nc.vector.tensor_scalar(tq[:], ps[:], ba, bq,
                                                        op0=ALU.mult,
                                                        op1=ALU.add)
                                nc.vector.scalar_tensor_tensor(
                                    stg[:, s, h0:h0 + hb, :],
                                    ps[:], bb, tq[:],
                                    op0=ALU.add, op1=ALU.max,
                                )
                    nc.scalar.dma_start(od[:], stg[:].rearrange(
                        "p s h w -> p (s h w)"))

                # ---------------- Phase A: uvx + uvy ----------------
                nc.tensor.matmul(warm_t[0:120, 0:120], wA_t[:, 0, :],
                                 wA_t[:, 0, :], start=True, stop=True)
                for vc in range(2):
                    cfg = VC_A[vc]
                    ncc = cfg["ncc"]
                    nvp = len(cfg["vps"])
                    M = 40 * nvp
                    xch = [xa_tiles[(vc, cc)] for cc in range(ncc)]
                    t0, kp0 = xch[0]
                    nc.tensor.matmul(warm_t[0:M, 0:256],
                                     wA_t[0:kp0, 0, 0:M],
                                     t0[:, 0:4, 0:64],
                                     start=True, stop=True)
                    for br in range(2):
                        stg = stgA_pool.tile([M, 64, 64], BF, tag="stgA",
                                             name=f"stgA{rep}_{vc}{br}")
                        unit = br * 2 + vc
                        ba = bias_t[0:M, 3 * unit + 2:3 * unit + 3]
                        bq = bias_t[0:M, 3 * unit + 1:3 * unit + 2]
                        bb = bias_t[0:M, 3 * unit:3 * unit + 1]
                        for g in range(8):
                            shape = [M, 8, 64]
                            ps = ps_pool.tile(shape, F32, tag="ps",
                                              name=f"psA{rep}_{vc}{br}{g}")
                            for cc in range(ncc):
                                t, kp = xch[cc]
                                for tap in range(KS):
                                    n = br * 30 + (tap * 6 + cc if vc == 0
                                                   else 18 + tap * 4 + cc)
                                    lhsT = wA_t[0:kp, n, 0:M]
                                    if br == 0:  # uvx: h-window shift
                                        rhs = t[0:kp,
                                                8 * g + tap:8 * g + tap + 8,
                                                1:65]
                                    else:  # uvy: shift along w
                                        rhs = t[0:kp, 1 + 8 * g:9 + 8 * g,
                                                tap:tap + 64]
                                    nc.tensor.matmul(
                                        ps[:], lhsT, rhs,
                                        start=(cc == 0 and tap == 0),
                                        stop=(cc == ncc - 1 and tap == 2),
                                    )
                            tq = tq_pool.tile(shape, F32, tag="tq",
                                              name=f"tqA{rep}_{vc}{br}{g}")
                            nc.vector.tensor_scalar(
                                tq[:], ps[:], ba, bq,
                                op0=ALU.mult, op1=ALU.add)
                            dst = stg[:, 8 * g:8 * g + 8, :]
                            nc.vector.scalar_tensor_tensor(
                                dst, ps[:], bb, tq[:],
                                op0=ALU.add, op1=ALU.max)
                            if g in (3, 7):
                                # write each branch's output in two halves
                                # (h 0:32 at g==3, 32:64 at g==7) so the
                                # end-of-NEFF tail is one 0.3MB DMA; vc1
                                # (the last phase-A chunk) rides the idle
                                # HWDGEs, vc0 the gpsimd SWDGE.
                                vp0 = cfg["vps"][0]
                                h0, h1 = (0, 32) if g == 3 else (32, 64)
                                dstd = out_d[br * 8:br * 8 + 8, :,
                                             vp0:vp0 + nvp, h0:h1, :]
                                if vc == 0:
                                    dge = nc.gpsimd
                                else:
                                    dge = nc.sync if br == 0 else nc.scalar
                                dge.dma_start(
                                    dstd.rearrange(
                                        "o u v h w -> (o u) v (h w)"),
                                    stg[:, h0:h1, :],
                                )

            # keep the warm/touch matmuls live: read a sliver out to dbg
            with tc.tile_pool(name="dbg", bufs=1) as dbg_pool:
                dbg_t = dbg_pool.tile([128, 4], F32)
                nc.vector.tensor_copy(dbg_t[:], warm_t[:, 0:4])
                nc.sync.dma_start(dbg_d[:], dbg_t[:])

    nc.compile()
    return nc


def _get_nc(repeat=1):
    global _NC_CACHE
    if _NC_CACHE is None:
        _NC_CACHE = {}
    if repeat not in _NC_CACHE:
        _NC_CACHE[repeat] = _build_nc(repeat)
    return _NC_CACHE[repeat]


# ---------------------------------------------------------------------------
# Entry point
# ---------------------------------------------------------------------------

LAST_RESULT = None


def _make_in_maps(x, w_uvx, b_uvx, a_uvx, w_uvy, b_uvy, a_uvy,
                  w_uxy, b_uxy, a_uxy, w_vxy, b_vxy, a_vxy):
    x = np.ascontiguousarray(np.asarray(x, np.float32))
    wA = _build_wA(np.asarray(w_uvx, np.float32), np.asarray(w_uvy, np.float32))
    wB = _build_wBC(np.asarray(w_uxy, np.float32))
    wC = _build_wBC(np.asarray(w_vxy, np.float32))
    bias = _build_bias(
        (b_uvx, b_uvy, b_uxy, b_vxy),
        [float(np.asarray(a).reshape(-1)[0]) for a in (a_uvx, a_uvy, a_uxy, a_vxy)],
    )
    in_maps = []
    for b in range(N_CORES):
        xb_full = x[b]
        xa1, xa2 = _prep_xa(xb_full)
        in_maps.append({
            "xa1": xa1.reshape(6, 120, 66 * 66),
            "xa2": xa2.reshape(4, 120, 66 * 66),
            "xb": _prep_xbc(xb_full, 0).reshape(4, 120, 5 * 66 * 23),
            "xc": _prep_xbc(xb_full, 1).reshape(4, 120, 5 * 66 * 23),
            "wA": wA, "wB": wB, "wC": wC, "bias": bias,
        })
    return in_maps


def kernel(x, w_uvx, b_uvx, a_uvx, w_uvy, b_uvy, a_uvy,
           w_uxy, b_uxy, a_uxy, w_vxy, b_vxy, a_vxy, _trace=False):
    from concourse.bass_utils import run_bass_kernel_spmd

    global LAST_RESULT
    in_maps = _make_in_maps(x, w_uvx, b_uvx, a_uvx, w_uvy, b_uvy, a_uvy,
                            w_uxy, b_uxy, a_uxy, w_vxy, b_vxy, a_vxy)

    nc = _get_nc()
    res = run_bass_kernel_spmd(nc, in_maps, core_ids=list(range(N_CORES)),
                               trace=_trace)
    LAST_RESULT = res

    full = np.empty((N_CORES, 32, U, V, H, W), np.float32)
    for b, r in enumerate(res.results):
        full[b, 0:16] = np.asarray(r["out"], np.float32)
        # raw B/C: (o, s', jw, fold, h', wb) -> w = 3*wb + jw
        for ch0, key in ((16, "outB"), (24, "outC")):
            raw = np.asarray(r[key], np.float32).reshape(8, 5, 3, 5, 64, 22)
            t = np.moveaxis(raw, 2, 5)  # (o, s', fold, h', wb, jw)
            asm = np.ascontiguousarray(t).reshape(8, 5, 5, 64, 66)[..., :64]
            if ch0 == 24:  # vxy: (o, v', u, h, w) -> (o, u, v', h, w)
                asm = asm.transpose(0, 2, 1, 3, 4)
            full[b, ch0:ch0 + 8] = asm
    return full

